# revision 11
# baseline (speedup 1.0000x reference)
"""MoE SwiGLU feed-forward (top-2 of 8 experts) on 8 Trainium2 NeuronCores.

Expert-parallel with host-side routing (the gate is tiny: 134 MFLOP on the
host vs 283 GFLOP of expert FFN on the device):
  host: exact fp32 gating -> top-2 ids + renormalized combine weights,
        per-expert token lists padded to a common tile-rounded cap,
        per-expert gathered+transposed bf16 token block xgT [D, cap]
        laid out for contiguous per-partition DMA.
  core e: streams expert e's weights (bf16, SBUF-resident, ~135KB/part)
        and its token block, then per 512-token block runs the SwiGLU FFN
        entirely on the PE in bf16 (feature-major layout, no on-device
        transposes, no indirect DMA), scaling by the combine weight on
        PSUM eviction, and writes a dense [cap, D] fp32 partial.
  host: out[idx_e] += y_e  (each token lands in exactly 2 expert lists).
"""

import sys

for p in ("/opt/trn_rl_repo", "/root/.axon_site/_ro/trn_rl_repo"):
    if p not in sys.path:
        sys.path.insert(0, p)

import numpy as np
import ml_dtypes

import concourse.bass as bass
import concourse.mybir as mybir
import concourse.tile as tile
from concourse import bacc
from concourse.bass_utils import run_bass_kernel_spmd
from concourse.masks import make_identity

P = 128
D = 1024          # model dim
H = 2816          # ffn hidden dim
E = 8             # experts == cores
T = 8192          # tokens
DC = D // P       # 8 contraction chunks
JCH = H // P      # 22 hidden chunks
BT = 512          # tokens per FFN block

f32 = mybir.dt.float32
bf16 = mybir.dt.bfloat16

_CACHE: dict = {}
RUN_KWARGS: dict = {}   # test hook: extra kwargs for run_bass_kernel_spmd
LAST_RESULT = None      # test hook: BassKernelResults of the last run


def _build(cap: int):
    tiles = cap // P
    nc = bacc.Bacc(None, target_bir_lowering=False, name="moe_hostroute")

    xg = nc.dram_tensor("xg", [P, DC * cap], bf16, kind="ExternalInput")
    # wg/wu host-packed jc-major: [p, ((jc*DC)+dc)*P + j] = w[jc*P+j, dc*P+p]
    wga = nc.dram_tensor("wga", [P, JCH * DC * P], bf16, kind="ExternalInput")
    wua = nc.dram_tensor("wua", [P, JCH * DC * P], bf16, kind="ExternalInput")
    wdT = nc.dram_tensor("wdT", [H, D], bf16, kind="ExternalInput")
    gat_d = nc.dram_tensor("gat", [P, tiles], f32, kind="ExternalInput")
    y = nc.dram_tensor("y", [cap, D], f32, kind="ExternalOutput")

    with tile.TileContext(nc) as tc:
        with (
            tc.tile_pool(name="keep", bufs=1) as keep,
            tc.tile_pool(name="xv", bufs=2) as xvp,
            tc.tile_pool(name="hts", bufs=1) as htsp,
            tc.tile_pool(name="sg", bufs=2) as sgp,
            tc.tile_pool(name="ysb", bufs=2) as ysbp,
            tc.tile_pool(name="wps", bufs=1, space="PSUM") as wpsp,
            tc.tile_pool(name="pgu", bufs=4, space="PSUM") as pgup,
            tc.tile_pool(name="pyp", bufs=3, space="PSUM") as pyp,
        ):
            identb = keep.tile([P, P], bf16, name="identb")
            make_identity(nc, identb[:])
            gat = keep.tile([P, tiles], f32, name="gat")
            nc.gpsimd.dma_start(gat[:], gat_d.ap())

            # wg/wu stream per-jc chunk (256KB each) on the two HWDGE
            # rings so block 0's matmuls start as soon as chunk 0 lands
            # and the stream stays ahead of the PE (~150 GB/s needed).
            wgs = keep.tile([P, JCH, DC * P], bf16, name="wgs")
            wus = keep.tile([P, JCH, DC * P], bf16, name="wus")
            wgc = wga.ap().rearrange("p (jc r) -> p jc r", jc=JCH)
            wuc = wua.ap().rearrange("p (jc r) -> p jc r", jc=JCH)
            for jc in range(JCH):
                nc.sync.dma_start(wgs[:, jc, :], wgc[:, jc, :])
                nc.scalar.dma_start(wus[:, jc, :], wuc[:, jc, :])

            # token blocks stream on the SWDGE ring (independent of the
            # weight streams on the two HWDGE rings)
            xcols = xg.ap().rearrange("p (dc t) -> p dc t", dc=DC)
            blocks = []
            c0 = 0
            while c0 < cap:
                blocks.append((c0, min(BT, cap - c0)))
                c0 += BT

            def load_block(bi):
                c0, tb = blocks[bi]
                xv = xvp.tile([P, DC, tb], bf16, name="xv")
                nc.gpsimd.dma_start(xv[:], xcols[:, :, c0:c0 + tb])
                return xv

            xv = load_block(0)
            xv_next = load_block(1) if len(blocks) > 1 else None

            # wd follows the token blocks on the SWDGE ring; it is not
            # needed until ~75us in, and later xv loads queue behind it
            # harmlessly (block period ~113us).
            wds = keep.tile([P, JCH, D], bf16, name="wds")
            nc.gpsimd.dma_start(wds[:], wdT.ap().rearrange("(jc p) d -> p jc d", p=P))

            # keep the PE busy (HAM warm-up) until chunk 0 lands
            wps = wpsp.tile([P, P], f32, name="wps")
            for _ in range(24):
                nc.tensor.matmul(wps[:], identb[:], identb[:],
                                 start=True, stop=True)

            for bi, (c0, tb) in enumerate(blocks):
                nt = tb // P
                # h = silu(x @ wg) * (x @ wu), one 128-chunk of hidden at a time
                hts = htsp.tile([P, JCH, tb], bf16, name="hts")
                for jc in range(JCH):
                    pg = pgup.tile([P, tb], f32, name="pg", tag="gu")
                    pu = pgup.tile([P, tb], f32, name="pu", tag="gu")
                    for dc in range(DC):
                        nc.tensor.matmul(
                            pg[:], wgs[:, jc, dc * P:(dc + 1) * P], xv[:, dc, :],
                            start=(dc == 0), stop=(dc == DC - 1),
                        )
                    for dc in range(DC):
                        nc.tensor.matmul(
                            pu[:], wus[:, jc, dc * P:(dc + 1) * P], xv[:, dc, :],
                            start=(dc == 0), stop=(dc == DC - 1),
                        )
                    sg = sgp.tile([P, tb], f32, name="sg")
                    nc.scalar.activation(sg[:], pg[:], mybir.ActivationFunctionType.Silu)
                    nc.vector.tensor_mul(hts[:, jc, :], sg[:], pu[:])
                # prefetch the next block's tokens behind this block's matmuls
                xv = xv_next
                if bi + 2 < len(blocks):
                    xv_next = load_block(bi + 2)
                # y = (h @ wd) * combine_weight, per 128-token tile
                for tt in range(nt):
                    g = c0 // P + tt
                    ysb = ysbp.tile([P, D], f32, name="ysb")
                    for ddh in range(2):
                        py = pyp.tile([P, 512], f32, name="py")
                        for jc in range(JCH):
                            nc.tensor.matmul(
                                py[:], hts[:, jc, tt * P:(tt + 1) * P],
                                wds[:, jc, ddh * 512:(ddh + 1) * 512],
                                start=(jc == 0), stop=(jc == JCH - 1),
                            )
                        nc.scalar.activation(
                            ysb[:, ddh * 512:(ddh + 1) * 512], py[:],
                            mybir.ActivationFunctionType.Copy,
                            scale=gat[:, g:g + 1],
                        )
                    nc.sync.dma_start(y.ap()[g * P:(g + 1) * P, :], ysb[:])

    nc.compile()
    return nc


def kernel(x, gate_w, wg, wu, wd):
    xf = np.ascontiguousarray(np.asarray(x, dtype=np.float32).reshape(T, D))
    gw = np.asarray(gate_w, dtype=np.float32)
    wg = np.asarray(wg, dtype=np.float32)
    wu = np.asarray(wu, dtype=np.float32)
    wd = np.asarray(wd, dtype=np.float32)

    # exact fp32 routing on the host
    logits = xf @ gw.T
    m = logits.max(axis=1, keepdims=True)
    sc = np.exp(logits - m)
    sc /= sc.sum(axis=1, keepdims=True)
    top2 = np.argpartition(-sc, 2, axis=1)[:, :2]
    tw = np.take_along_axis(sc, top2, axis=1)
    order = np.argsort(-tw, axis=1)
    top2 = np.take_along_axis(top2, order, axis=1)
    tw = np.take_along_axis(tw, order, axis=1)
    tw = tw / tw.sum(axis=1, keepdims=True)

    idxs, wts = [], []
    for e in range(E):
        sel = (top2 == e)
        rows = np.where(sel.any(axis=1))[0]
        w = (tw * sel[:, :2])[rows].sum(axis=1)
        idxs.append(rows)
        wts.append(w.astype(np.float32))
    cap = max(128, -(-max(len(r) for r in idxs) // P) * P)
    tiles = cap // P
    if cap not in _CACHE:
        _CACHE[cap] = _build(cap)
    nc = _CACHE[cap]

    xbf = xf.astype(ml_dtypes.bfloat16)

    def pack_jc(w):
        # [H, D] -> [P, JCH*DC*P] with [p, ((jc*DC)+dc)*P + j] = w[jc*P+j, dc*P+p]
        return np.ascontiguousarray(
            w.reshape(JCH, P, DC, P).transpose(3, 0, 2, 1).reshape(P, JCH * DC * P)
        ).astype(ml_dtypes.bfloat16)

    in_maps = []
    for e in range(E):
        idx, w = idxs[e], wts[e]
        n = len(idx)
        # gathered+transposed token block: xgT[d, t] = x[idx[t], d],
        # packed as [P, DC*cap] with column dc*cap + t = row dc*128+p of xgT
        xgT = np.zeros((D, cap), dtype=ml_dtypes.bfloat16)
        xgT[:, :n] = xbf[idx].T
        xgn = np.ascontiguousarray(
            xgT.reshape(DC, P, cap).transpose(1, 0, 2).reshape(P, DC * cap))
        gflat = np.zeros(cap, dtype=np.float32)
        gflat[:n] = w                         # slot g*128+p <-> (p, g)
        gatn = np.ascontiguousarray(gflat.reshape(tiles, P).T)
        in_maps.append({
            "xg": xgn,
            "gat": gatn,
            "wga": pack_jc(wg[e]),
            "wua": pack_jc(wu[e]),
            "wdT": np.ascontiguousarray(wd[e].T).astype(ml_dtypes.bfloat16),
        })
    res = run_bass_kernel_spmd(nc, in_maps, core_ids=list(range(E)), **RUN_KWARGS)
    globals()["LAST_RESULT"] = res
    out = np.zeros((T, D), dtype=np.float32)
    for e in range(E):
        n = len(idxs[e])
        out[idxs[e]] += res.results[e]["y"][:n]
    return out.reshape(np.asarray(x).shape)
